# revision 1
# baseline (speedup 1.0000x reference)
"""Trainium2 Bass kernel for nn_BoneRefusion (17-group BoneMLP over [B,T,16,3]).

Strategy (pure data parallel over batch, 8 cores):
  - Host pre-packs per-core inputs into a feature-major, "2-set" layout:
      xT2 [98, S] bf16, S = tokens_per_core/2.
      Rows 0-47 = 48 features (16 bones x 3 coords) of token set A (first half),
      row 48 = ones (bakes b1 into the layer-1 matmul), rows 49-96 = set B,
      row 97 = ones. Column j holds the token pair (A_j, B_j).
  - Layer 1 (h = relu(x @ W1 + b1)) runs as 5 matmul passes w=0..4 with
    block-diagonal stationary weights [98, Mw] (Mw=128 for w<4: 64 h-features
    x 2 sets; w=4: 16 features x 2 sets). PSUM rows = h features interleaved
    by set. Evacuation PSUM->SBUF applies ReLU and casts to bf16.
  - Layer 2 (out = h @ W2 + b2) runs as column-tiled matmuls (tile_position)
    so 3 of them stream concurrently; b2 is added during PSUM evacuation.
  - Output leaves the device feature-major; the host transposes it back.

All matmuls are bf16 (fp32 matmul is 4x slower on the PE; error measured at
~2e-3 relative on this problem). Accumulation and output are fp32.
"""

import sys

import numpy as np
import ml_dtypes

sys.path.insert(0, "/opt/trn_rl_repo")

import concourse.bass as bass
import concourse.mybir as mybir
import concourse.tile as tile
from concourse import bacc
from concourse.bass_utils import run_bass_kernel_spmd

BF16 = mybir.dt.bfloat16
F32 = mybir.dt.float32
BF16_NP = ml_dtypes.bfloat16

LIMBS = [[0, 1, 2], [3, 4, 5], [6, 7], [8, 9], [10, 11, 12], [13, 14, 15],
         [6, 7, 1, 2], [6, 7, 4, 5], [6, 7, 11, 12], [6, 7, 14, 15], [6, 7, 9],
         [14, 15, 11, 12], [1, 2, 4, 5], [14, 15, 4, 5], [11, 12, 4, 5],
         [10, 0], [13, 3]]
NG = 17          # groups
HID = 16         # hidden per group
B, T, NJ, C = 2048, 243, 16, 3
NF = NJ * C      # 48 input features per token
NCORES = 8
BC = B // NCORES           # batches per core
TC = BC * T                # tokens per core
S = TC // 2                # token pairs per core (2-set packing)
KX = 2 * (NF + 1)          # 98: two sets of (48 features + ones row)
NBLK = 512                 # token-pairs per inner iteration (psum free dim)

# layer-1 passes: groups per pass, features per pass (x2 sets in M)
PASS_GROUPS = [(0, 4), (4, 4), (8, 4), (12, 4), (16, 1)]  # (first group, count)


def _host_weights(W1, b1, W2, b2, idx):
    """Build the stationary operands on the host.

    Returns (w1l [98, 640] bf16, w2l [128, 160] bf16, b2a [96] f32, b2b [56] f32).
    """
    W1 = np.asarray(W1, np.float32)
    b1 = np.asarray(b1, np.float32)
    W2 = np.asarray(W2, np.float32)
    b2 = np.asarray(b2, np.float32)
    idx = np.asarray(idx)

    # Scatter the per-group [12, 16] W1 blocks into the 48-feature space.
    # Padded limb rows of W1 are already zero, so += handles duplicates.
    w1full = np.zeros((NF, NG * HID), np.float32)
    for g in range(NG):
        for j in range(4):
            r = int(idx[g, j]) * C
            w1full[r:r + C, g * HID:(g + 1) * HID] += W1[g, j * C:(j + 1) * C, :]
    b1flat = b1.reshape(NG * HID)

    # Layer-1 stationary tiles, one [98, 128] block per pass (pass 4: [98, 32]).
    w1l = np.zeros((KX, 5 * 128), np.float32)
    for w, (g0, ng) in enumerate(PASS_GROUPS):
        m = ng * HID
        blk = w1full[:, g0 * HID:(g0 + ng) * HID]      # [48, m]
        bias = b1flat[g0 * HID:(g0 + ng) * HID]        # [m]
        col = w * 128
        w1l[0:NF, col:col + m] = blk                   # set A weights
        w1l[NF, col:col + m] = bias                    # set A bias (ones row)
        w1l[NF + 1:2 * NF + 1, col + m:col + 2 * m] = blk   # set B
        w1l[2 * NF + 1, col + m:col + 2 * m] = bias

    # Layer-2 stationary tiles [128, 32] per pass (pass 4 uses rows 0-31).
    # h_sb tile rows: 0..m-1 = set A features, 64..64+m-1 (or m..2m-1 for w=4)
    # = set B features. Output cols: 0..3*ng-1 set A, 12..12+3*ng-1 set B for
    # w<4 (M padded to 32 with zero cols); w=4: cols 0-2 A, 3-5 B.
    w2l = np.zeros((128, 5 * 32), np.float32)
    for w, (g0, ng) in enumerate(PASS_GROUPS):
        col = w * 32
        m = ng * HID
        boff = 64 if w < 4 else m                     # set-B row offset in h tile
        cb = 12 if w < 4 else C * ng                  # set-B col offset
        for j in range(ng):
            g = g0 + j
            w2l[16 * j:16 * j + 16, col + 3 * j:col + 3 * j + 3] = W2[g]
            w2l[boff + 16 * j:boff + 16 * j + 16,
                col + cb + 3 * j:col + cb + 3 * j + 3] = W2[g]

    # Evacuation biases, per psum partition.
    # slot A psum rows: q0: w=0 (rows 0-11 A g0-3, 12-23 B g0-3, 24-31 zero),
    # q1: w=1 (+32), q2: w=2 (+64).  slot B: q0: w=3, q1: w=4 (rows 32-34 A
    # g16, 35-37 B g16).
    b2a = np.zeros(96, np.float32)
    for q, (g0, ng) in enumerate(PASS_GROUPS[:3]):
        v = b2[g0:g0 + ng].reshape(-1)                # 12 values
        b2a[32 * q:32 * q + 12] = v
        b2a[32 * q + 12:32 * q + 24] = v
    b2b = np.zeros(56, np.float32)
    v = b2[12:16].reshape(-1)
    b2b[0:12] = v
    b2b[12:24] = v
    b2b[32:35] = b2[16]
    b2b[35:38] = b2[16]

    return (w1l.astype(BF16_NP), w2l.astype(BF16_NP), b2a[:, None], b2b[:, None])


def _build_nc(repeat=1):
    nc = bacc.Bacc(
        "TRN2", target_bir_lowering=False, debug=False, num_devices=NCORES,
    )
    x2 = nc.dram_tensor("x2", [KX, S], BF16, kind="ExternalInput").ap()
    w1 = nc.dram_tensor("w1", [KX, 5 * 128], BF16, kind="ExternalInput").ap()
    w2 = nc.dram_tensor("w2", [128, 5 * 32], BF16, kind="ExternalInput").ap()
    b2a = nc.dram_tensor("b2a", [96, 1], F32, kind="ExternalInput").ap()
    b2b = nc.dram_tensor("b2b", [56, 1], F32, kind="ExternalInput").ap()
    # Device output, feature-major: slot A rows 0-87 (g0-11 + pad rows) and
    # slot B rows 0-37 (g12-16 + pad rows), concatenated: [126, S].
    outd = nc.dram_tensor("outd", [126, S], F32, kind="ExternalOutput").ap()

    with tile.TileContext(nc) as tc:
        with (
            tc.tile_pool(name="singles", bufs=1) as singles,
            tc.tile_pool(name="xin", bufs=3) as xin,
            tc.tile_pool(name="hsb", bufs=2) as hsb,
            tc.tile_pool(name="osb", bufs=2) as osb,
            tc.tile_pool(name="hps", bufs=1, space="PSUM") as hps,
            tc.tile_pool(name="ops", bufs=1, space="PSUM") as opsp,
        ):
            w1_sb = singles.tile([KX, 5, 128], BF16)
            nc.sync.dma_start(w1_sb, w1.rearrange("k (w m) -> k w m", w=5))
            w2_sb = singles.tile([128, 5, 32], BF16)
            nc.sync.dma_start(w2_sb, w2.rearrange("k (w m) -> k w m", w=5))
            b2a_sb = singles.tile([96, 1], F32)
            nc.sync.dma_start(b2a_sb, b2a)
            b2b_sb = singles.tile([56, 1], F32)
            nc.sync.dma_start(b2b_sb, b2b)

            for _rep in range(repeat):
              off = 0
              while off < S:
                nb = min(NBLK, S - off)
                xt = xin.tile([KX, NBLK], BF16, tag="xt")
                nc.sync.dma_start(xt[:, :nb], x2[:, off:off + nb])

                # ---- layer 1: two double-bank psum tiles + one single ----
                hp01 = hps.tile([128, 2, NBLK], F32, tag="hp01")
                hp23 = hps.tile([128, 2, NBLK], F32, tag="hp23")
                hp4 = hps.tile([32, NBLK], F32, tag="hp4")
                hts = []
                for w in range(5):
                    if w < 4:
                        dst = (hp01 if w < 2 else hp23)[:, w % 2, :nb]
                    else:
                        dst = hp4[:, :nb]
                    mw = 128 if w < 4 else 32
                    nc.tensor.matmul(
                        dst[:mw] if w < 4 else dst,
                        lhsT=w1_sb[:, w, :mw],
                        rhs=xt[:, :nb],
                        start=True, stop=True,
                    )
                # relu evacuations (cast to bf16)
                h01 = hsb.tile([128, 2, NBLK], BF16, tag="h01")
                h23 = hsb.tile([128, 2, NBLK], BF16, tag="h23")
                h4 = hsb.tile([32, NBLK], BF16, tag="h4")
                nc.scalar.activation(
                    out=h01[:, :, :nb], in_=hp01[:, :, :nb],
                    func=mybir.ActivationFunctionType.Relu,
                )
                nc.vector.tensor_scalar(
                    h23[:, :, :nb], hp23[:, :, :nb], 0.0, None,
                    mybir.AluOpType.max,
                )
                nc.vector.tensor_scalar(
                    h4[:, :nb], hp4[:, :nb], 0.0, None,
                    mybir.AluOpType.max,
                )
                hts = [h01[:, 0], h01[:, 1], h23[:, 0], h23[:, 1], h4]

                # ---- layer 2: column-tiled matmuls ----
                oa = opsp.tile([128, NBLK], F32, tag="oa")
                ob = opsp.tile([64, NBLK], F32, tag="ob")
                for w in range(3):
                    nc.tensor.matmul(
                        oa[32 * w:32 * w + 32, :nb],
                        lhsT=w2_sb[:, w, :],
                        rhs=hts[w][:, :nb],
                        start=True, stop=True,
                        tile_position=(0, 32 * w),
                    )
                nc.tensor.matmul(
                    ob[0:32, :nb], lhsT=w2_sb[:, 3, :], rhs=hts[3][:, :nb],
                    start=True, stop=True, tile_position=(0, 0),
                )
                nc.tensor.matmul(
                    ob[32:64, :nb], lhsT=w2_sb[:32, 4, :], rhs=hts[4][:, :nb],
                    start=True, stop=True, tile_position=(0, 32),
                )

                # ---- out evacuations with b2 bias ----
                oas = osb.tile([96, NBLK], F32, tag="oas")
                obs = osb.tile([56, NBLK], F32, tag="obs")
                nc.scalar.activation(
                    out=oas[:, :nb], in_=oa[:96, :nb],
                    func=mybir.ActivationFunctionType.Identity,
                    bias=b2a_sb, scale=1.0,
                )
                nc.vector.tensor_scalar(
                    obs[:, :nb], ob[:56, :nb], b2b_sb, None,
                    mybir.AluOpType.add,
                )

                # ---- store (pad rows included; host slices them off) ----
                nc.sync.dma_start(outd[0:88, off:off + nb], oas[:88, :nb])
                nc.sync.dma_start(outd[88:126, off:off + nb], obs[:38, :nb])

                off += nb
    nc.finalize()
    return nc


def _bench_pair(reps=60, repeats=(1, 5)):
    """Measure kernel time via internal-repeat slope; returns ns per kernel."""
    import time
    import jax
    from jax.sharding import Mesh, PartitionSpec, NamedSharding
    from jax.experimental.shard_map import shard_map
    from concourse import bass2jax
    from concourse.bass2jax import _bass_exec_p, install_neuronx_cc_hook

    install_neuronx_cc_hook()
    rng = np.random.default_rng(0)
    times = {}
    for rep in repeats:
        nc = _build_nc(repeat=rep)
        in_names, out_names, out_avals, zero_outs = [], [], [], []
        for alloc in nc.m.functions[0].allocations:
            if not isinstance(alloc, mybir.MemoryLocationSet):
                continue
            name = alloc.memorylocations[0].name
            if alloc.kind == "ExternalInput":
                if (nc.partition_id_tensor is not None
                        and name == nc.partition_id_tensor.name):
                    continue
                in_names.append(name)
            elif alloc.kind == "ExternalOutput":
                shape = tuple(alloc.tensor_shape)
                dt = mybir.dt.np(alloc.dtype)
                out_avals.append(jax.core.ShapedArray(shape, dt))
                out_names.append(name)
                zero_outs.append(np.zeros(shape, dt))
        n_params, n_outs = len(in_names), len(out_names)
        bind_names = list(in_names) + list(out_names)
        if nc.partition_id_tensor is not None:
            bind_names.append(nc.partition_id_tensor.name)

        def _body(*args, _nc=nc, _oa=tuple(out_avals), _bn=tuple(bind_names),
                  _on=tuple(out_names)):
            operands = list(args)
            if _nc.partition_id_tensor is not None:
                operands.append(bass2jax.partition_id_tensor())
            return tuple(_bass_exec_p.bind(
                *operands, out_avals=_oa, in_names=_bn, out_names=_on,
                lowering_input_output_aliases=(), sim_require_finite=True,
                sim_require_nnan=True, nc=_nc))

        devices = jax.devices()[:NCORES]
        mesh = Mesh(np.asarray(devices), ("core",))
        in_specs = (PartitionSpec("core"),) * (n_params + n_outs)
        out_specs = (PartitionSpec("core"),) * n_outs
        donate = tuple(range(n_params, n_params + n_outs))
        fn = jax.jit(shard_map(_body, mesh=mesh, in_specs=in_specs,
                               out_specs=out_specs, check_rep=False),
                     donate_argnums=donate, keep_unused=True)
        sh = NamedSharding(mesh, PartitionSpec("core"))
        shapes = {"x2": (KX, S), "w1": (KX, 640), "w2": (128, 160),
                  "b2a": (96, 1), "b2b": (56, 1)}
        dts = {"x2": BF16_NP, "w1": BF16_NP, "w2": BF16_NP,
               "b2a": np.float32, "b2b": np.float32}
        concat_in = [jax.device_put(
            rng.normal(size=(NCORES * shapes[nm][0], *shapes[nm][1:])
                       ).astype(dts[nm]) * 0.1, sh) for nm in in_names]

        def make_zeros():
            zs = [jax.device_put(
                np.zeros((NCORES * z.shape[0], *z.shape[1:]), z.dtype), sh)
                for z in zero_outs]
            for a in zs:
                a.block_until_ready()
            return zs

        outs = fn(*concat_in, *make_zeros())
        jax.block_until_ready(outs)
        best = None
        for _trial in range(3):
            zsets = [make_zeros() for _ in range(reps)]
            t0 = time.time()
            for r in range(reps):
                outs = fn(*concat_in, *zsets[r])
            jax.block_until_ready(outs)
            dt = (time.time() - t0) / reps
            best = dt if best is None else min(best, dt)
        times[rep] = best
        print(f"repeat={rep}: {best * 1e6:.1f} us/call", flush=True)
    r0, r1 = repeats
    tk = (times[r1] - times[r0]) / (r1 - r0)
    print(f"kernel time (slope): {tk * 1e9:.0f} ns", flush=True)
    return tk * 1e9


_NC_CACHE = None


def _get_nc():
    global _NC_CACHE
    if _NC_CACHE is None:
        _NC_CACHE = _build_nc()
    return _NC_CACHE


def _kernel_impl(x, W1, b1, W2, b2, idx, _want_trace=False):
    x = np.asarray(x, np.float32)
    w1l, w2l, b2a, b2b = _host_weights(W1, b1, W2, b2, idx)

    in_maps = []
    for c in range(NCORES):
        xc = x[c * BC:(c + 1) * BC].reshape(TC, NF)
        xt2 = np.empty((KX, S), BF16_NP)
        xt2[0:NF] = np.ascontiguousarray(xc[:S].T)
        xt2[NF] = np.float32(1.0)
        xt2[NF + 1:2 * NF + 1] = np.ascontiguousarray(xc[S:].T)
        xt2[2 * NF + 1] = np.float32(1.0)
        in_maps.append({
            "x2": xt2, "w1": w1l, "w2": w2l, "b2a": b2a, "b2b": b2b,
        })

    nc = _get_nc()
    res = run_bass_kernel_spmd(
        nc, in_maps, core_ids=list(range(NCORES)), trace=_want_trace,
    )

    out = np.empty((B, T, NG, C), np.float32)
    for c in range(NCORES):
        od = res.results[c]["outd"]            # [126, S]
        # row map: slot A quarters at 0/32/64 (12 set-A rows then 12 set-B
        # rows each, then 8 pad); slot B at 88 (g12-15) and 120 (g16).
        rows_a = np.r_[0:12, 32:44, 64:76, 88:100, 120:123]
        rows_b = rows_a + np.r_[[12] * 48, [3] * 3]
        oc = np.empty((TC, NG * C), np.float32)
        oc[:S] = od[rows_a].T
        oc[S:] = od[rows_b].T
        out[c * BC:(c + 1) * BC] = oc.reshape(BC, T, NG, C)
    return out, res


def kernel(**inputs):
    out, _ = _kernel_impl(**inputs)
    return out



# revision 5
# speedup vs baseline: 1.1422x; 1.1422x over previous
"""Trainium2 Bass kernel for nn_BoneRefusion (17-group BoneMLP over [B,T,16,3]).

Strategy (pure data parallel over batch, 8 cores):
  - Host pre-packs per-core inputs feature-major in a 2-set layout:
      x2 [96, S] bf16, S = tokens_per_core/2.
      Rows 0-47 = 48 features (16 bones x 3 coords) of token set A (first
      half), rows 48-95 = set B. Column j holds the token pair (A_j, B_j).
  - Layer 1 (h = relu(x @ W1 + b1)): 4 full-width matmul passes w=0..3 with
    block-diagonal stationary [96, 128] (64 h-features x 2 sets each) plus a
    narrow pass p4 [96, 32] for group 16. b1 is added during the PSUM
    evacuation (fused with ReLU and the bf16 cast), not via a ones-row.
  - Layer 2 (out = h @ W2 + b2): five 32-column streams, column-tiled so four
    run concurrently in one PE pass. Software-pipelined: step s computes
    L1 of block s and L2 of block s-1, so L2's semaphore waits (on h
    evacuation) are long satisfied and the PE never stalls.
  - PE round structure per step (each round = 128 PE columns, N=512):
      r0-r3: L1 p0..p3 (full width)
      r4:    L2 g12-15 | g0-3 | g4-7 | g8-11   (4x 32-col, concurrent)
      r5:    L1 p4 | L2 g16                    (2x 32-col, concurrent)
  - Output leaves the device feature-major in bf16 (tolerance is 2e-2;
    measured error ~2.5e-3); the host transposes/casts back to f32.

All matmuls are bf16; PSUM accumulation fp32 (TRN2 requires fp32 PSUM).
"""

import sys

import numpy as np
import ml_dtypes

sys.path.insert(0, "/opt/trn_rl_repo")

import concourse.bass as bass
import concourse.mybir as mybir
import concourse.tile as tile
from concourse import bacc
from concourse.bass_utils import run_bass_kernel_spmd

BF16 = mybir.dt.bfloat16
F32 = mybir.dt.float32
BF16_NP = ml_dtypes.bfloat16

LIMBS = [[0, 1, 2], [3, 4, 5], [6, 7], [8, 9], [10, 11, 12], [13, 14, 15],
         [6, 7, 1, 2], [6, 7, 4, 5], [6, 7, 11, 12], [6, 7, 14, 15], [6, 7, 9],
         [14, 15, 11, 12], [1, 2, 4, 5], [14, 15, 4, 5], [11, 12, 4, 5],
         [10, 0], [13, 3]]
NG = 17          # groups
HID = 16         # hidden per group
B, T, NJ, C = 2048, 243, 16, 3
NF = NJ * C      # 48 input features per token
NCORES = 8
BC = B // NCORES           # batches per core
TC = BC * T                # tokens per core
S = TC // 2                # token pairs per core (2-set packing)
KX = 2 * NF                # 96 contraction rows (two sets of 48 features)
NBLK = 512                 # token-pairs per block (psum free dim)
NB = (S + NBLK - 1) // NBLK   # 61 blocks (60x512 + 1x384)

# L2 stream order across PSUM quarters of the `op` bank: stream q covers
# GROUPS_L2[q], reading h of L1 pass PASS_OF_STREAM[q] from the prev block.
GROUPS_L2 = [(12, 4), (0, 4), (4, 4), (8, 4)]
PASS_OF_STREAM = [3, 0, 1, 2]


def _host_weights(W1, b1, W2, b2, idx):
    """Build stationary operands + evac bias vectors on the host.

    Returns (wsb [128, 704] bf16, bsb [128, 7] f32).
      wsb cols 0-511: L1 passes 0-3 ([96,128] each: rows 0-47 set A block,
        rows 48-95 set B block, both for the same 64 h-features).
      wsb cols 512-639: L2 streams q=0..3 ([128,32] each).
      wsb cols 640-671: L1 p4 (group 16 hidden, [96,32]).
      wsb cols 672-703: L2 g16 ([32,32]).
      bsb col w (0-3): b1 for pass w (64 feats x 2 sets).
      bsb col 4: b2 for the L2 psum bank (per-partition).
      bsb col 5: hx-bank bias (rows 0-31: b1 g16; rows 32-37: b2 g16).
      bsb col 6: hx-bank relu mask (0 on h rows, -1e30 on out rows).
    """
    W1 = np.asarray(W1, np.float32)
    b1 = np.asarray(b1, np.float32)
    W2 = np.asarray(W2, np.float32)
    b2 = np.asarray(b2, np.float32)
    idx = np.asarray(idx)

    # Scatter per-group [12, 16] W1 blocks into the 48-feature space.
    # Padded limb rows of W1 are already zero, so += handles duplicates.
    w1full = np.zeros((NF, NG * HID), np.float32)
    for g in range(NG):
        for j in range(4):
            r = int(idx[g, j]) * C
            w1full[r:r + C, g * HID:(g + 1) * HID] += W1[g, j * C:(j + 1) * C, :]
    b1flat = b1.reshape(NG * HID)

    wsb = np.zeros((128, 704), np.float32)
    for w in range(4):
        blk = w1full[:, 64 * w:64 * w + 64]            # [48, 64]
        wsb[0:NF, 128 * w:128 * w + 64] = blk          # set A
        wsb[NF:2 * NF, 128 * w + 64:128 * w + 128] = blk   # set B
    for q, (g0, ng) in enumerate(GROUPS_L2):
        col = 512 + 32 * q
        for j in range(ng):
            g = g0 + j
            wsb[16 * j:16 * j + 16, col + 3 * j:col + 3 * j + 3] = W2[g]
            wsb[64 + 16 * j:64 + 16 * j + 16,
                col + 12 + 3 * j:col + 12 + 3 * j + 3] = W2[g]
    wsb[0:NF, 640:656] = w1full[:, 256:272]            # p4 set A
    wsb[NF:2 * NF, 656:672] = w1full[:, 256:272]       # p4 set B
    wsb[0:16, 672:675] = W2[16]                        # g16 L2 set A
    wsb[16:32, 675:678] = W2[16]                       # g16 L2 set B

    bsb = np.zeros((128, 7), np.float32)
    for w in range(4):
        bsb[0:64, w] = b1flat[64 * w:64 * w + 64]
        bsb[64:128, w] = b1flat[64 * w:64 * w + 64]
    for q, (g0, ng) in enumerate(GROUPS_L2):
        v = b2[g0:g0 + ng].reshape(-1)                 # 12 values
        bsb[32 * q:32 * q + 12, 4] = v
        bsb[32 * q + 12:32 * q + 24, 4] = v
    bsb[0:16, 5] = b1flat[256:272]
    bsb[16:32, 5] = b1flat[256:272]
    bsb[32:35, 5] = b2[16]
    bsb[35:38, 5] = b2[16]
    bsb[32:64, 6] = -1e30                              # identity rows of hx

    return wsb.astype(BF16_NP), bsb


def _build_nc():
    nc = bacc.Bacc(
        "TRN2", target_bir_lowering=False, debug=False, num_devices=NCORES,
    )
    x2 = nc.dram_tensor("x2", [KX, S], BF16, kind="ExternalInput").ap()
    wsd = nc.dram_tensor("wsd", [128, 704], BF16, kind="ExternalInput").ap()
    bsd = nc.dram_tensor("bsd", [128, 7], F32, kind="ExternalInput").ap()
    # Device output, feature-major bf16: rows 0-127 = L2 psum bank layout
    # (quarter q rows 32q..32q+24 real), rows 128-135 = g16 out (+2 pad).
    outd = nc.dram_tensor("outd", [136, S], BF16, kind="ExternalOutput").ap()

    with tile.TileContext(nc) as tc:
        with (
            tc.tile_pool(name="singles", bufs=1) as singles,
            tc.tile_pool(name="xin", bufs=3) as xin,
            tc.tile_pool(name="hsb", bufs=2) as hsb,
            tc.tile_pool(name="hxsb", bufs=2) as hxsb,
            tc.tile_pool(name="osb", bufs=3) as osb,
            tc.tile_pool(name="hps", bufs=1, space="PSUM") as hps,
            tc.tile_pool(name="ops", bufs=1, space="PSUM") as opsp,
            tc.tile_pool(name="hxps", bufs=1, space="PSUM") as hxps,
        ):
            ws = singles.tile([128, 704], BF16)
            nc.sync.dma_start(ws, wsd)
            bs = singles.tile([128, 7], F32)
            nc.sync.dma_start(bs, bsd)

            h_prev = None       # [h0..h3] sbuf tiles of previous block
            hx_prev = None      # hx sbuf tile of previous block
            nb_prev = 0

            for s in range(NB + 1):
                cur = s if s < NB else None
                prev = s - 1 if s >= 1 else None
                xt = None
                hp = None
                if cur is not None:
                    off = cur * NBLK
                    nb = min(NBLK, S - off)
                    xt = xin.tile([KX, NBLK], BF16, tag="xt")
                    nc.sync.dma_start(xt[:, :nb], x2[:, off:off + nb])

                # ---- PE rounds ----
                if cur is not None:
                    hp = [hps.tile([128, NBLK], F32, tag=f"hp{w}",
                                   name=f"hp{w}") for w in range(4)]
                    for w in range(4):
                        nc.tensor.matmul(
                            hp[w][:, :nb],
                            lhsT=ws[0:KX, 128 * w:128 * w + 128],
                            rhs=xt[:, :nb],
                            start=True, stop=True,
                        )
                if prev is not None:
                    op = opsp.tile([128, NBLK], F32, tag="op")
                    for q in range(4):
                        nc.tensor.matmul(
                            op[32 * q:32 * q + 32, :nb_prev],
                            lhsT=ws[0:128, 512 + 32 * q:512 + 32 * q + 32],
                            rhs=h_prev[PASS_OF_STREAM[q]][:, :nb_prev],
                            start=True, stop=True,
                            tile_position=(0, 32 * q),
                        )
                hx = hxps.tile([128, NBLK], F32, tag="hx")
                if cur is not None:
                    nc.tensor.matmul(
                        hx[0:32, :nb], lhsT=ws[0:KX, 640:672],
                        rhs=xt[:, :nb], start=True, stop=True,
                        tile_position=(0, 0),
                    )
                if prev is not None:
                    nc.tensor.matmul(
                        hx[32:64, :nb_prev], lhsT=ws[0:32, 672:704],
                        rhs=hx_prev[0:32, :nb_prev], start=True, stop=True,
                        tile_position=(0, 32),
                    )

                # ---- evacuations ----
                if cur is not None:
                    hts = [hsb.tile([128, NBLK], BF16, tag=f"h{w}",
                                    name=f"h{w}") for w in range(4)]
                    for w in range(4):
                        if w % 2 == 0:
                            nc.scalar.activation(
                                out=hts[w][:, :nb], in_=hp[w][:, :nb],
                                func=mybir.ActivationFunctionType.Relu,
                                bias=bs[:, w:w + 1], scale=1.0,
                            )
                        else:
                            nc.vector.tensor_scalar(
                                hts[w][:, :nb], hp[w][:, :nb],
                                bs[:, w:w + 1], 0.0,
                                mybir.AluOpType.add, mybir.AluOpType.max,
                            )
                else:
                    hts = None
                # hx bank: rows 0-31 relu(h16 + b1), rows 32-39 out + b2
                hxs = hxsb.tile([64, NBLK], BF16, tag="hxs")
                if cur is not None:
                    nc.vector.tensor_scalar(
                        hxs[0:32, :nb], hx[0:32, :nb], bs[0:32, 5:6], 0.0,
                        mybir.AluOpType.add, mybir.AluOpType.max,
                    )
                if prev is not None:
                    nc.scalar.activation(
                        out=hxs[32:40, :nb_prev], in_=hx[32:40, :nb_prev],
                        func=mybir.ActivationFunctionType.Identity,
                        bias=bs[32:40, 5:6], scale=1.0,
                    )
                if prev is not None:
                    ost = osb.tile([128, NBLK], BF16, tag="os")
                    nc.scalar.activation(
                        out=ost[:, :nb_prev], in_=op[:, :nb_prev],
                        func=mybir.ActivationFunctionType.Identity,
                        bias=bs[:, 4:5], scale=1.0,
                    )
                    offp = prev * NBLK
                    nc.sync.dma_start(
                        outd[0:128, offp:offp + nb_prev], ost[:, :nb_prev])
                    nc.sync.dma_start(
                        outd[128:136, offp:offp + nb_prev],
                        hxs[32:40, :nb_prev])

                h_prev = hts
                hx_prev = hxs
                if cur is not None:
                    nb_prev = nb
    nc.finalize()
    return nc


_NC_CACHE = None


def _get_nc():
    global _NC_CACHE
    if _NC_CACHE is None:
        _NC_CACHE = _build_nc()
    return _NC_CACHE


# outd row map (see _build_nc): stream q of GROUPS_L2 at rows 32q..32q+24
# (12 set-A cols then 12 set-B), g16 at rows 128-133.  In group order 0..16:
_ROWS_A = np.r_[32:44, 64:76, 96:108, 0:12, 128:131]
_ROWS_B = np.r_[44:56, 76:88, 108:120, 12:24, 131:134]


def _kernel_impl(x, W1, b1, W2, b2, idx, _want_trace=False):
    x = np.asarray(x, np.float32)
    wsb, bsb = _host_weights(W1, b1, W2, b2, idx)

    in_maps = []
    for c in range(NCORES):
        xc = x[c * BC:(c + 1) * BC].reshape(TC, NF)
        xt2 = np.empty((KX, S), BF16_NP)
        xt2[0:NF] = np.ascontiguousarray(xc[:S].T)
        xt2[NF:2 * NF] = np.ascontiguousarray(xc[S:].T)
        in_maps.append({"x2": xt2, "wsd": wsb, "bsd": bsb})

    nc = _get_nc()
    res = run_bass_kernel_spmd(
        nc, in_maps, core_ids=list(range(NCORES)), trace=_want_trace,
    )

    out = np.empty((B, T, NG, C), np.float32)
    for c in range(NCORES):
        od = np.asarray(res.results[c]["outd"], dtype=np.float32)  # [136, S]
        oc = np.empty((TC, NG * C), np.float32)
        oc[:S] = od[_ROWS_A].T
        oc[S:] = od[_ROWS_B].T
        out[c * BC:(c + 1) * BC] = oc.reshape(BC, T, NG, C)
    return out, res


def kernel(**inputs):
    out, _ = _kernel_impl(**inputs)
    return out
